# revision 1
# baseline (speedup 1.0000x reference)
"""LorentzKG scoring kernel for 8 Trainium2 NeuronCores. v4.

Engine schedule (chunk j, pair q = chunks (2q, 2q+1)):
  DVE  iter j: big(j) = rotation+boost+tt0+b1 -> v_rs,v_big
               reduce(j-1) (after ACT square)
               even j: tailA(q=(j-2)/2) -> v_ic ; tailB1(q=(j-4)/2) -> v_u ;
                       tailB2(q=(j-6)/2) -> v_done     (pairs, 2K wide)
  GPSIMD j:    res = ns + w ; pd = res * t_sp  (after v_rs(j)) -> g_pd
  ACT j:       square(j) (after g_pd(j)) -> a_sq ; pair sqrt / ln+square
  sync:        H,R prefetch after v_big; T prefetch after g_pd; pair stores
"""
import numpy as np

import concourse.bass as bass
import concourse.mybir as mybir
from concourse.bass_utils import run_bass_kernel_spmd

NE = 1_000_000
NR = 1000
D = 32
B = 1_048_576
NCORES = 8
BCORE = B // NCORES          # 131072
P = 128
K = 64
CHUNK = P * K                # 8192
NCH = BCORE // CHUNK         # 16
NPAIR = NCH // 2
HW = 34
RW = 68

TRACE = False
LAST_EXEC_NS = None
DBG_OUT = None

_NC_CACHE = []

F32 = mybir.dt.float32
MUL = mybir.AluOpType.mult
ADD = mybir.AluOpType.add
SUB = mybir.AluOpType.subtract
MAX = mybir.AluOpType.max


def _build_nc():
    nc = bass.Bass()
    h_in = nc.declare_dram_parameter("h", [BCORE, HW], F32, isOutput=False)
    t_in = nc.declare_dram_parameter("t", [BCORE, HW], F32, isOutput=False)
    r_in = nc.declare_dram_parameter("r", [BCORE, RW], F32, isOutput=False)
    cst_in = nc.declare_dram_parameter("cst", [P, 8], F32, isOutput=False)
    out = nc.declare_dram_parameter("out", [BCORE], F32, isOutput=True)

    h_d = h_in[:].rearrange("(c p k) d -> c p (k d)", p=P, k=K)
    t_d = t_in[:].rearrange("(c p k) d -> c p (k d)", p=P, k=K)
    r_d = r_in[:].rearrange("(c p k) d -> c p (k d)", p=P, k=K)
    o_d2 = out[:].rearrange("(q c p k) -> q p c k", c=2, p=P, k=K)

    ctx_list = []

    def sb(width):
        cm = nc.sbuf_tensor([P, width], F32)
        t = cm.__enter__()
        ctx_list.append(cm)
        return t

    cst_sb = sb(8)
    h_sb = sb(2 * K * HW)
    t_sb = sb(2 * K * HW)
    r_sb = sb(2 * K * RW)
    ns_sb = sb(2 * K * 32)
    sq_sb = sb(2 * K * 32)
    pp_sb = sb(2 * K * 32)        # P1 rotation temp, then PD (gpsimd)
    o_sb = sb(4 * K)              # 2 pairs
    xb_sb = sb(2 * K)             # boost temps (chunk, 2-slot)
    cb_sb = sb(2 * K * 16)        # gpsimd rotation product C*b
    tb_sb = sb(2 * K)
    # chunk-indexed 4-slot smalls (pair reads use adjacent slots)
    r2_sb = sb(4 * K)
    dot_sb = sb(4 * K)
    tt0_sb = sb(4 * K)
    b1_sb = sb(8 * K)             # long lifetime (read at iter 2q+6)
    # pair-indexed 2-slot smalls, 2K wide each
    pr = {n: sb(2 * 2 * K) for n in
          ["time", "xs", "t0c", "u1", "ic", "qm", "s2", "u", "dd"]}

    sems = {}
    for n in ["in_sem", "outst", "c_sem", "v_big", "v_rs", "v_ic", "v_u",
              "v_done", "a_sq", "a_s2", "a_dd", "g_pd", "g_cb"]:
        cm = nc.semaphore(n)
        sems[n] = cm.__enter__()
        ctx_list.append(cm)

    def view(t, j, width, d):
        s = j % 2
        return t[:, s * K * width:(s + 1) * K * width].rearrange(
            "p (k d) -> p k d", d=d)

    def hv(j):
        return view(h_sb, j, HW, HW)

    def tv(j):
        return view(t_sb, j, HW, HW)

    def rv(j):
        return view(r_sb, j, RW, RW)

    def nsv(j):
        return view(ns_sb, j, 32, 32)

    def sqv(j):
        return view(sq_sb, j, 32, 32)

    def ppv(j):
        return view(pp_sb, j, 32, 32)

    def ch4(t, j):   # chunk-indexed 4-slot [P, K] view
        s = j % 4
        return t[:, s * K:(s + 1) * K]

    def ch4p(t, q):  # pair view over adjacent slots (2q)%4, (2q)%4+1
        s = (2 * q) % 4
        return t[:, s * K:(s + 2) * K]

    def b1v(j):
        s = j % 8
        return b1_sb[:, s * K:(s + 1) * K]

    def b1p(q):
        s = (2 * q) % 8
        return b1_sb[:, s * K:(s + 2) * K]

    def prv(n, q):   # pair-indexed [P, 2K] view
        s = q % 2
        return pr[n][:, s * 2 * K:(s + 1) * 2 * K]

    def opv(q):      # out pair view
        s = q % 2
        return o_sb[:, s * 2 * K:(s + 1) * 2 * K]

    blk_cm = nc.Block()
    blk = blk_cm.__enter__()

    @blk.sync
    def _(sync):
        sync.dma_start(out=cst_sb[:, 0:8], in_=cst_in[:]).then_inc(
            sems["c_sem"], 16)
        for j in range(min(2, NCH)):
            sync.dma_start(out=hv(j), in_=h_d[j]).then_inc(sems["in_sem"], 16)
            sync.dma_start(out=tv(j), in_=t_d[j]).then_inc(sems["in_sem"], 16)
            sync.dma_start(out=rv(j), in_=r_d[j]).then_inc(sems["in_sem"], 16)
        for j in range(NCH):
            if j + 2 < NCH:
                sync.wait_ge(sems["v_big"], j + 1)
                sync.dma_start(out=hv(j + 2), in_=h_d[j + 2]).then_inc(
                    sems["in_sem"], 16)
                sync.dma_start(out=rv(j + 2), in_=r_d[j + 2]).then_inc(
                    sems["in_sem"], 16)
                sync.wait_ge(sems["g_pd"], j + 1)
                sync.dma_start(out=tv(j + 2), in_=t_d[j + 2]).then_inc(
                    sems["in_sem"], 16)
            if j >= 7 and (j - 7) % 2 == 0:
                q = (j - 7) // 2
                sync.wait_ge(sems["v_done"], q + 1)
                sync.dma_start(out=o_d2[q], in_=opv(q)).then_inc(
                    sems["outst"], 16)
        for q in range((NCH - 7 + 1) // 2, NPAIR):
            sync.wait_ge(sems["v_done"], q + 1)
            sync.dma_start(out=o_d2[q], in_=opv(q)).then_inc(sems["outst"], 16)

    @blk.vector
    def _(vector):
        tt = nc.vector.tensor_tensor

        def cb_(i, w=K):
            return cst_sb[:, i:i + 1].to_broadcast([P, w])

        vector.wait_ge(sems["c_sem"], 16)
        for j in range(NCH + 5):
            if j < NCH:
                H, T, R = hv(j), tv(j), rv(j)
                NS, PP = nsv(j), ppv(j)
                vector.wait_ge(sems["in_sem"], 48 * (j + 1))
                tt(out=PP[:, :, :], in0=R[:, :, 0:32], in1=H[:, :, 1:33],
                   op=MUL)
                tt(out=NS[:, :, 0:16], in0=PP[:, :, 0:16],
                   in1=PP[:, :, 16:32], op=SUB)
                tt(out=PP[:, :, 0:16], in0=R[:, :, 16:32], in1=H[:, :, 1:17],
                   op=MUL)
                vector.wait_ge(sems["g_cb"], j + 1)
                tt(out=NS[:, :, 16:32], in0=PP[:, :, 0:16],
                   in1=view(cb_sb, j, 16, 16)[:, :, :], op=ADD)
                tt(out=xb_sb[:, (j % 2) * K:(j % 2 + 1) * K],
                   in0=H[:, :, 0], in1=R[:, :, 64], op=MUL)
                tt(out=tb_sb[:, (j % 2) * K:(j % 2 + 1) * K],
                   in0=NS[:, :, 0], in1=R[:, :, 65], op=MUL)
                tt(out=NS[:, :, 0],
                   in0=tb_sb[:, (j % 2) * K:(j % 2 + 1) * K],
                   in1=xb_sb[:, (j % 2) * K:(j % 2 + 1) * K], op=ADD)
                tt(out=NS[:, :, :], in0=NS[:, :, :], in1=R[:, :, 32:64],
                   op=ADD)
                tt(out=ch4(tt0_sb, j), in0=T[:, :, 0], in1=cb_(3), op=MUL)
                tt(out=b1v(j), in0=H[:, :, 33], in1=T[:, :, 33], op=ADD)
                vector.drain()
                vector.sem_inc(sems["v_rs"], 1)
                vector.sem_inc(sems["v_big"], 1)
            jr = j - 1
            if 0 <= jr < NCH:
                vector.wait_ge(sems["a_sq"], jr + 1)
                vector.wait_ge(sems["g_pd"], jr + 1)
                nc.vector.reduce_sum(out=ch4(r2_sb, jr), in_=sqv(jr)[:, :, :],
                                     axis=mybir.AxisListType.X)
                nc.vector.reduce_sum(out=ch4(dot_sb, jr),
                                     in_=ppv(jr)[:, :, :],
                                     axis=mybir.AxisListType.X)
            if j >= 2 and j % 2 == 0:
                q = (j - 2) // 2
                if q < NPAIR:
                    x = ch4p(r2_sb, q)
                    W = 2 * K
                    tt(out=prv("xs", q), in0=x, in1=cb_(0, W), op=MUL)
                    tt(out=prv("xs", q), in0=prv("xs", q), in1=cb_(1, W),
                       op=ADD)
                    tt(out=prv("t0c", q), in0=prv("xs", q), in1=x, op=MUL)
                    tt(out=prv("t0c", q), in0=prv("t0c", q), in1=cb_(2, W),
                       op=ADD)
                    tt(out=prv("time", q), in0=prv("t0c", q), in1=x, op=MUL)
                    tt(out=prv("time", q), in0=prv("time", q), in1=cb_(3, W),
                       op=ADD)
                    tt(out=prv("u1", q), in0=prv("time", q),
                       in1=ch4p(tt0_sb, q), op=MUL)
                    tt(out=prv("u1", q), in0=prv("u1", q),
                       in1=ch4p(dot_sb, q), op=SUB)
                    tt(out=prv("ic", q), in0=prv("u1", q), in1=cb_(4, W),
                       op=MAX)
                    tt(out=prv("xs", q), in0=prv("ic", q), in1=cb_(3, W),
                       op=SUB)
                    tt(out=prv("t0c", q), in0=prv("ic", q), in1=cb_(3, W),
                       op=ADD)
                    tt(out=prv("qm", q), in0=prv("xs", q), in1=prv("t0c", q),
                       op=MUL)
                    vector.drain()
                    vector.sem_inc(sems["v_ic"], 1)
            if j >= 4 and j % 2 == 0:
                q = (j - 4) // 2
                if q < NPAIR:
                    vector.wait_ge(sems["a_s2"], q + 1)
                    tt(out=prv("u", q), in0=prv("ic", q), in1=prv("s2", q),
                       op=ADD)
                    vector.drain()
                    vector.sem_inc(sems["v_u"], 1)
            if j >= 6 and j % 2 == 0:
                q = (j - 6) // 2
                if q < NPAIR:
                    if q >= 2:
                        vector.wait_ge(sems["outst"], 16 * (q - 1))
                    vector.wait_ge(sems["a_dd"], q + 1)
                    if DBG_OUT is None:
                        tt(out=opv(q), in0=b1p(q), in1=prv("dd", q), op=SUB)
                    else:
                        nc.vector.tensor_copy(out=opv(q),
                                              in_=prv(DBG_OUT, q))
                    vector.drain()
                    vector.sem_inc(sems["v_done"], 1)

    @blk.scalar
    def _(scalar):
        act = nc.scalar.activation
        AF = mybir.ActivationFunctionType
        for j in range(NCH + 5):
            if j < NCH:
                scalar.wait_ge(sems["v_rs"], j + 1)
                act(out=sqv(j)[:, :, :], in_=nsv(j)[:, :, :], func=AF.Square)
                scalar.drain()
                scalar.sem_inc(sems["a_sq"], 1)
            if j >= 2 and j % 2 == 0:
                q = (j - 2) // 2
                if q < NPAIR:
                    scalar.wait_ge(sems["v_ic"], q + 1)
                    act(out=prv("s2", q), in_=prv("qm", q), func=AF.Sqrt)
                    scalar.drain()
                    scalar.sem_inc(sems["a_s2"], 1)
            if j >= 4 and j % 2 == 0:
                q = (j - 4) // 2
                if q < NPAIR:
                    scalar.wait_ge(sems["v_u"], q + 1)
                    act(out=prv("dd", q), in_=prv("u", q), func=AF.Ln)
                    act(out=prv("dd", q), in_=prv("dd", q), func=AF.Square)
                    scalar.drain()
                    scalar.sem_inc(sems["a_dd"], 1)

    @blk.gpsimd
    def _(gpsimd):
        for j in range(NCH):
            gpsimd.wait_ge(sems["in_sem"], 48 * (j + 1))
            nc.gpsimd.tensor_tensor(
                out=view(cb_sb, j, 16, 16)[:, :, :], in0=rv(j)[:, :, 0:16],
                in1=hv(j)[:, :, 17:33], op=MUL)
            gpsimd.drain()
            gpsimd.sem_inc(sems["g_cb"], 1)
            gpsimd.wait_ge(sems["v_rs"], j + 1)
            nc.gpsimd.tensor_tensor(
                out=ppv(j)[:, :, :], in0=nsv(j)[:, :, :],
                in1=tv(j)[:, :, 1:33], op=MUL)
            gpsimd.drain()
            gpsimd.sem_inc(sems["g_pd"], 1)

    blk_cm.__exit__(None, None, None)
    nc._ctx_keepalive = ctx_list
    return nc


def _get_nc():
    if not _NC_CACHE:
        _NC_CACHE.append(_build_nc())
    return _NC_CACHE[0]


def _host_pack(heads, relations, tails, entity_emb, rel_boost_w, rel_rot_w,
               rel_trans_w, ent_bias_w):
    heads = np.asarray(heads).astype(np.int64)
    relations = np.asarray(relations).astype(np.int64)
    tails = np.asarray(tails).astype(np.int64)
    entity_emb = np.asarray(entity_emb, dtype=np.float32)
    ent_bias_w = np.asarray(ent_bias_w, dtype=np.float32)

    rot = np.asarray(rel_rot_w, dtype=np.float32).astype(np.float64)
    boost = np.asarray(rel_boost_w, dtype=np.float32).astype(np.float64)
    trans = np.asarray(rel_trans_w, dtype=np.float32).astype(np.float64)

    c = np.cos(rot[:, :16])
    s = np.sin(rot[:, :16])
    rap0 = np.clip(boost[:, 0], -2.0, 2.0)
    c0 = np.cosh(rap0)
    s0 = np.sinh(rap0)
    tv = 0.1 * trans
    vn = np.sqrt(np.clip(np.sum(tv * tv, axis=1), 1e-6, None))
    cvn = np.cosh(vn)
    w = (np.sinh(vn) / vn)[:, None] * tv

    rel_packed = np.zeros((NR, RW), dtype=np.float32)
    rel_packed[:, 0:16] = (cvn[:, None] * c).astype(np.float32)
    rel_packed[:, 16:32] = (cvn[:, None] * s).astype(np.float32)
    rel_packed[:, 32:64] = w.astype(np.float32)
    rel_packed[:, 64] = (cvn * s0).astype(np.float32)
    rel_packed[:, 65] = c0.astype(np.float32)

    ent_packed = np.concatenate([entity_emb, ent_bias_w], axis=1)

    h_stream = ent_packed[heads]
    t_stream = ent_packed[tails]
    r_stream = rel_packed[relations]
    return h_stream, t_stream, r_stream


def kernel(heads, relations, tails, entity_emb, rel_boost_w, rel_rot_w,
           rel_trans_w, ent_bias_w):
    global LAST_EXEC_NS
    h_stream, t_stream, r_stream = _host_pack(
        heads, relations, tails, entity_emb, rel_boost_w, rel_rot_w,
        rel_trans_w, ent_bias_w)

    nc = _get_nc()
    cst = np.zeros((P, 8), dtype=np.float32)
    cst[:, 0] = 0.0625
    cst[:, 1] = -0.125
    cst[:, 2] = 0.5
    cst[:, 3] = 1.0
    cst[:, 4] = 1.0 + 1e-6
    in_maps = []
    for i in range(NCORES):
        sl = slice(i * BCORE, (i + 1) * BCORE)
        in_maps.append({"h": np.ascontiguousarray(h_stream[sl]),
                        "t": np.ascontiguousarray(t_stream[sl]),
                        "r": np.ascontiguousarray(r_stream[sl]),
                        "cst": cst})

    res = run_bass_kernel_spmd(nc, in_maps, core_ids=list(range(NCORES)),
                               trace=TRACE)
    LAST_EXEC_NS = res.exec_time_ns
    return np.concatenate([res.results[i]["out"] for i in range(NCORES)])



# revision 4
# speedup vs baseline: 1.2828x; 1.2828x over previous
"""LorentzKG scoring kernel for 8 Trainium2 NeuronCores. v5 (bf16).

Streams host-gathered per-element rows in bf16 (h 34, t 34, r 66) so the
big DVE tensor_tensor ops run in 2x mode and HBM traffic halves vs f32.
Row layouts keep hot 16/32-wide slices 4B-aligned (2x-mode requirement):
  h row: [sp(32), x0, b_h+b_t]        t row: [sp(32), t0-1, pad]
  r row: [cvn*cos(16), cvn*sin(16), w(32), cvn*s0, c0]

Engine schedule (chunk j, pair q = chunks (2q, 2q+1)):
  DVE  iter j: rotation+boost+w -> NS (bf16 2x), tt0, b1 copy
               reduce(j-1) of SQ/PD (f32, 1x)
               even j: tailA(q=(j-2)/2) ; tailB1(q=(j-4)/2) ; tailB2(q=(j-6)/2)
  GPSIMD j:    cb = Ccvn*b (rotation cross term) ; PD = NS * t_sp
  ACT j:       SQ = NS^2 ; pair sqrt / ln+square
  sync:        H,R prefetch after v_big; T prefetch after g_pd; pair stores
"""
import numpy as np
import ml_dtypes

import concourse.bass as bass
import concourse.mybir as mybir
from concourse.bass_utils import run_bass_kernel_spmd

NE = 1_000_000
NR = 1000
D = 32
B = 1_048_576
NCORES = 8
BCORE = B // NCORES          # 131072
P = 128
K = 64
CHUNK = P * K                # 8192
NCH = BCORE // CHUNK         # 16
NPAIR = NCH // 2
HW = 34
RW = 66

TRACE = False
LAST_EXEC_NS = None
DBG_OUT = None

_NC_CACHE = []

F32 = mybir.dt.float32
BF16 = mybir.dt.bfloat16
MUL = mybir.AluOpType.mult
ADD = mybir.AluOpType.add
SUB = mybir.AluOpType.subtract
MAX = mybir.AluOpType.max


def _build_nc():
    nc = bass.Bass()
    h_in = nc.declare_dram_parameter("h", [BCORE, HW], BF16, isOutput=False)
    t_in = nc.declare_dram_parameter("t", [BCORE, HW], BF16, isOutput=False)
    r_in = nc.declare_dram_parameter("r", [BCORE, RW], BF16, isOutput=False)
    cst_in = nc.declare_dram_parameter("cst", [P, 8], F32, isOutput=False)
    out = nc.declare_dram_parameter("out", [BCORE], F32, isOutput=True)

    h_d = h_in[:].rearrange("(c p k) d -> c p (k d)", p=P, k=K)
    t_d = t_in[:].rearrange("(c p k) d -> c p (k d)", p=P, k=K)
    r_d = r_in[:].rearrange("(c p k) d -> c p (k d)", p=P, k=K)
    o_d2 = out[:].rearrange("(q c p k) -> q p c k", c=2, p=P, k=K)

    ctx_list = []

    def sb(width, dt=F32):
        cm = nc.sbuf_tensor([P, width], dt)
        t = cm.__enter__()
        ctx_list.append(cm)
        return t

    cst_sb = sb(8)
    h_sb = sb(2 * K * HW, BF16)
    t_sb = sb(2 * K * HW, BF16)
    r_sb = sb(2 * K * RW, BF16)
    ns_sb = sb(2 * K * 32, BF16)
    pp_sb = sb(2 * K * 32, BF16)  # rotation temp
    sq_sb = sb(2 * K * 32)        # ACT square out (f32)
    pd_sb = sb(2 * K * 32)        # gpsimd NS*t_sp out (f32)
    o_sb = sb(4 * K)              # 2 pairs
    xb_sb = sb(2 * K, BF16)       # boost temps (chunk, 2-slot)
    cb_sb = sb(2 * K * 16, BF16)  # gpsimd rotation product Ccvn*b
    tb_sb = sb(2 * K, BF16)
    # chunk-indexed 4-slot smalls (pair reads use adjacent slots)
    r2_sb = sb(4 * K)
    dot_sb = sb(4 * K)
    tt0_sb = sb(4 * K)
    b1_sb = sb(8 * K)             # long lifetime (read at iter 2q+6)
    # pair-indexed 2-slot smalls, 2K wide each
    pr = {n: sb(2 * 2 * K) for n in
          ["time", "xs", "t0c", "u1", "ic", "qm", "s2", "u", "dd"]}

    sems = {}
    for n in ["in_sem", "outst", "c_sem", "v_big", "v_rs", "v_ic", "v_u",
              "v_done", "a_sq", "a_s2", "a_dd", "g_pd", "g_cb"]:
        cm = nc.semaphore(n)
        sems[n] = cm.__enter__()
        ctx_list.append(cm)

    def view(t, j, width, d):
        s = j % 2
        return t[:, s * K * width:(s + 1) * K * width].rearrange(
            "p (k d) -> p k d", d=d)

    def hv(j):
        return view(h_sb, j, HW, HW)

    def tv(j):
        return view(t_sb, j, HW, HW)

    def rv(j):
        return view(r_sb, j, RW, RW)

    def nsv(j):
        return view(ns_sb, j, 32, 32)

    def ppv(j):
        return view(pp_sb, j, 32, 32)

    def sqv(j):
        return view(sq_sb, j, 32, 32)

    def pdv(j):
        return view(pd_sb, j, 32, 32)

    def ch4(t, j):   # chunk-indexed 4-slot [P, K] view
        s = j % 4
        return t[:, s * K:(s + 1) * K]

    def ch4p(t, q):  # pair view over adjacent slots (2q)%4, (2q)%4+1
        s = (2 * q) % 4
        return t[:, s * K:(s + 2) * K]

    def b1v(j):
        s = j % 8
        return b1_sb[:, s * K:(s + 1) * K]

    def b1p(q):
        s = (2 * q) % 8
        return b1_sb[:, s * K:(s + 2) * K]

    def prv(n, q):   # pair-indexed [P, 2K] view
        s = q % 2
        return pr[n][:, s * 2 * K:(s + 1) * 2 * K]

    def opv(q):      # out pair view
        s = q % 2
        return o_sb[:, s * 2 * K:(s + 1) * 2 * K]

    blk_cm = nc.Block()
    blk = blk_cm.__enter__()

    @blk.sync
    def _(sync):
        sync.dma_start(out=cst_sb[:, 0:8], in_=cst_in[:]).then_inc(
            sems["c_sem"], 16)
        for j in range(min(2, NCH)):
            sync.dma_start(out=hv(j), in_=h_d[j]).then_inc(sems["in_sem"], 16)
            sync.dma_start(out=tv(j), in_=t_d[j]).then_inc(sems["in_sem"], 16)
            sync.dma_start(out=rv(j), in_=r_d[j]).then_inc(sems["in_sem"], 16)
        for j in range(NCH):
            if j + 2 < NCH:
                sync.wait_ge(sems["v_big"], j + 1)
                sync.dma_start(out=hv(j + 2), in_=h_d[j + 2]).then_inc(
                    sems["in_sem"], 16)
                sync.dma_start(out=rv(j + 2), in_=r_d[j + 2]).then_inc(
                    sems["in_sem"], 16)
                sync.wait_ge(sems["g_pd"], j + 1)
                sync.dma_start(out=tv(j + 2), in_=t_d[j + 2]).then_inc(
                    sems["in_sem"], 16)
            if j >= 7 and (j - 7) % 2 == 0:
                q = (j - 7) // 2
                sync.wait_ge(sems["v_done"], q + 1)
                sync.dma_start(out=o_d2[q], in_=opv(q)).then_inc(
                    sems["outst"], 16)
        for q in range((NCH - 7 + 1) // 2, NPAIR):
            sync.wait_ge(sems["v_done"], q + 1)
            sync.dma_start(out=o_d2[q], in_=opv(q)).then_inc(sems["outst"], 16)

    @blk.vector
    def _(vector):
        tt = nc.vector.tensor_tensor

        def cb_(i, w=K):
            return cst_sb[:, i:i + 1].to_broadcast([P, w])

        vector.wait_ge(sems["c_sem"], 16)
        for j in range(NCH + 5):
            if j < NCH:
                H, T, R = hv(j), tv(j), rv(j)
                NS, PP = nsv(j), ppv(j)
                vector.wait_ge(sems["in_sem"], 48 * (j + 1))
                # PP = [Ccvn*a | Scvn*b]  (bf16 2x)
                tt(out=PP[:, :, :], in0=R[:, :, 0:32], in1=H[:, :, 0:32],
                   op=MUL)
                # rot_lo = Ccvn*a - Scvn*b
                tt(out=NS[:, :, 0:16], in0=PP[:, :, 0:16],
                   in1=PP[:, :, 16:32], op=SUB)
                # Scvn*a
                tt(out=PP[:, :, 0:16], in0=R[:, :, 16:32], in1=H[:, :, 0:16],
                   op=MUL)
                vector.wait_ge(sems["g_cb"], j + 1)
                # rot_hi = Scvn*a + Ccvn*b
                tt(out=NS[:, :, 16:32], in0=PP[:, :, 0:16],
                   in1=view(cb_sb, j, 16, 16)[:, :, :], op=ADD)
                # boost on spatial component 0
                tt(out=xb_sb[:, (j % 2) * K:(j % 2 + 1) * K],
                   in0=H[:, :, 32], in1=R[:, :, 64], op=MUL)
                tt(out=tb_sb[:, (j % 2) * K:(j % 2 + 1) * K],
                   in0=NS[:, :, 0], in1=R[:, :, 65], op=MUL)
                tt(out=NS[:, :, 0],
                   in0=tb_sb[:, (j % 2) * K:(j % 2 + 1) * K],
                   in1=xb_sb[:, (j % 2) * K:(j % 2 + 1) * K], op=ADD)
                # exp-map tangent add (bf16 2x)
                tt(out=NS[:, :, :], in0=NS[:, :, :], in1=R[:, :, 32:64],
                   op=ADD)
                # tt0 = t0 = (t0-1) + 1   (f32 out)
                tt(out=ch4(tt0_sb, j), in0=T[:, :, 32], in1=cb_(3), op=ADD)
                # b1 = b_h + b_t (host-packed in h slot 33); cast to f32 ring
                # (tensor_copy with strided bf16 src misreads; tt is safe)
                tt(out=b1v(j), in0=H[:, :, 33], in1=cb_(5), op=ADD)
                vector.drain()
                vector.sem_inc(sems["v_rs"], 1)
                vector.sem_inc(sems["v_big"], 1)
            jr = j - 1
            if 0 <= jr < NCH:
                vector.wait_ge(sems["a_sq"], jr + 1)
                vector.wait_ge(sems["g_pd"], jr + 1)
                nc.vector.reduce_sum(out=ch4(r2_sb, jr), in_=sqv(jr)[:, :, :],
                                     axis=mybir.AxisListType.X)
                nc.vector.reduce_sum(out=ch4(dot_sb, jr),
                                     in_=pdv(jr)[:, :, :],
                                     axis=mybir.AxisListType.X)
            if j >= 2 and j % 2 == 0:
                q = (j - 2) // 2
                if q < NPAIR:
                    x = ch4p(r2_sb, q)
                    W = 2 * K
                    tt(out=prv("xs", q), in0=x, in1=cb_(0, W), op=MUL)
                    tt(out=prv("xs", q), in0=prv("xs", q), in1=cb_(1, W),
                       op=ADD)
                    tt(out=prv("t0c", q), in0=prv("xs", q), in1=x, op=MUL)
                    tt(out=prv("t0c", q), in0=prv("t0c", q), in1=cb_(2, W),
                       op=ADD)
                    tt(out=prv("time", q), in0=prv("t0c", q), in1=x, op=MUL)
                    tt(out=prv("time", q), in0=prv("time", q), in1=cb_(3, W),
                       op=ADD)
                    tt(out=prv("u1", q), in0=prv("time", q),
                       in1=ch4p(tt0_sb, q), op=MUL)
                    tt(out=prv("u1", q), in0=prv("u1", q),
                       in1=ch4p(dot_sb, q), op=SUB)
                    tt(out=prv("ic", q), in0=prv("u1", q), in1=cb_(4, W),
                       op=MAX)
                    tt(out=prv("xs", q), in0=prv("ic", q), in1=cb_(3, W),
                       op=SUB)
                    tt(out=prv("t0c", q), in0=prv("ic", q), in1=cb_(3, W),
                       op=ADD)
                    tt(out=prv("qm", q), in0=prv("xs", q), in1=prv("t0c", q),
                       op=MUL)
                    vector.drain()
                    vector.sem_inc(sems["v_ic"], 1)
            if j >= 4 and j % 2 == 0:
                q = (j - 4) // 2
                if q < NPAIR:
                    vector.wait_ge(sems["a_s2"], q + 1)
                    tt(out=prv("u", q), in0=prv("ic", q), in1=prv("s2", q),
                       op=ADD)
                    vector.drain()
                    vector.sem_inc(sems["v_u"], 1)
            if j >= 6 and j % 2 == 0:
                q = (j - 6) // 2
                if q < NPAIR:
                    if q >= 2:
                        vector.wait_ge(sems["outst"], 16 * (q - 1))
                    vector.wait_ge(sems["a_dd"], q + 1)
                    if DBG_OUT is None:
                        tt(out=opv(q), in0=b1p(q), in1=prv("dd", q), op=SUB)
                    else:
                        nc.vector.tensor_copy(out=opv(q),
                                              in_=prv(DBG_OUT, q))
                    vector.drain()
                    vector.sem_inc(sems["v_done"], 1)

    @blk.scalar
    def _(scalar):
        act = nc.scalar.activation
        AF = mybir.ActivationFunctionType
        for j in range(NCH + 5):
            if j < NCH:
                scalar.wait_ge(sems["v_rs"], j + 1)
                act(out=sqv(j)[:, :, :], in_=nsv(j)[:, :, :], func=AF.Square)
                scalar.drain()
                scalar.sem_inc(sems["a_sq"], 1)
            if j >= 2 and j % 2 == 0:
                q = (j - 2) // 2
                if q < NPAIR:
                    scalar.wait_ge(sems["v_ic"], q + 1)
                    act(out=prv("s2", q), in_=prv("qm", q), func=AF.Sqrt)
                    scalar.drain()
                    scalar.sem_inc(sems["a_s2"], 1)
            if j >= 4 and j % 2 == 0:
                q = (j - 4) // 2
                if q < NPAIR:
                    scalar.wait_ge(sems["v_u"], q + 1)
                    act(out=prv("dd", q), in_=prv("u", q), func=AF.Ln)
                    act(out=prv("dd", q), in_=prv("dd", q), func=AF.Square)
                    scalar.drain()
                    scalar.sem_inc(sems["a_dd"], 1)

    @blk.gpsimd
    def _(gpsimd):
        for j in range(NCH):
            gpsimd.wait_ge(sems["in_sem"], 48 * (j + 1))
            # cb = Ccvn * b  (rotation cross term)
            nc.gpsimd.tensor_tensor(
                out=view(cb_sb, j, 16, 16)[:, :, :], in0=rv(j)[:, :, 0:16],
                in1=hv(j)[:, :, 16:32], op=MUL)
            gpsimd.drain()
            gpsimd.sem_inc(sems["g_cb"], 1)
            gpsimd.wait_ge(sems["v_rs"], j + 1)
            # PD = NS * t_sp  (f32 out, reduced by DVE)
            nc.gpsimd.tensor_tensor(
                out=pdv(j)[:, :, :], in0=nsv(j)[:, :, :],
                in1=tv(j)[:, :, 0:32], op=MUL)
            gpsimd.drain()
            gpsimd.sem_inc(sems["g_pd"], 1)

    blk_cm.__exit__(None, None, None)
    nc._ctx_keepalive = ctx_list
    return nc


def _get_nc():
    if not _NC_CACHE:
        _NC_CACHE.append(_build_nc())
    return _NC_CACHE[0]


def _host_pack(heads, relations, tails, entity_emb, rel_boost_w, rel_rot_w,
               rel_trans_w, ent_bias_w):
    heads = np.asarray(heads).astype(np.int64)
    relations = np.asarray(relations).astype(np.int64)
    tails = np.asarray(tails).astype(np.int64)
    entity_emb = np.asarray(entity_emb, dtype=np.float32)
    ent_bias_w = np.asarray(ent_bias_w, dtype=np.float32)

    rot = np.asarray(rel_rot_w, dtype=np.float32).astype(np.float64)
    boost = np.asarray(rel_boost_w, dtype=np.float32).astype(np.float64)
    trans = np.asarray(rel_trans_w, dtype=np.float32).astype(np.float64)

    c = np.cos(rot[:, :16])
    s = np.sin(rot[:, :16])
    rap0 = np.clip(boost[:, 0], -2.0, 2.0)
    c0 = np.cosh(rap0)
    s0 = np.sinh(rap0)
    tv = 0.1 * trans
    vn = np.sqrt(np.clip(np.sum(tv * tv, axis=1), 1e-6, None))
    cvn = np.cosh(vn)
    w = (np.sinh(vn) / vn)[:, None] * tv

    rel_packed = np.zeros((NR, RW), dtype=ml_dtypes.bfloat16)
    rel_packed[:, 0:16] = (cvn[:, None] * c).astype(ml_dtypes.bfloat16)
    rel_packed[:, 16:32] = (cvn[:, None] * s).astype(ml_dtypes.bfloat16)
    rel_packed[:, 32:64] = w.astype(ml_dtypes.bfloat16)
    rel_packed[:, 64] = (cvn * s0).astype(ml_dtypes.bfloat16)
    rel_packed[:, 65] = c0.astype(ml_dtypes.bfloat16)

    # entity rows: [sp(32), x0, bias] ; tail variant uses x0-1 in slot 32
    x0 = entity_emb[:, 0:1]
    sp = entity_emb[:, 1:]
    hrow = np.zeros((NE, HW), dtype=ml_dtypes.bfloat16)
    hrow[:, 0:32] = sp.astype(ml_dtypes.bfloat16)
    hrow[:, 32] = x0[:, 0].astype(ml_dtypes.bfloat16)
    trow = np.zeros((NE, HW), dtype=ml_dtypes.bfloat16)
    trow[:, 0:32] = sp.astype(ml_dtypes.bfloat16)
    trow[:, 32] = (x0[:, 0] - 1.0).astype(ml_dtypes.bfloat16)

    h_stream = hrow[heads]
    h_stream[:, 33] = (ent_bias_w[heads, 0]
                       + ent_bias_w[tails, 0]).astype(ml_dtypes.bfloat16)
    t_stream = trow[tails]
    r_stream = rel_packed[relations]
    return h_stream, t_stream, r_stream


def kernel(heads, relations, tails, entity_emb, rel_boost_w, rel_rot_w,
           rel_trans_w, ent_bias_w):
    global LAST_EXEC_NS
    h_stream, t_stream, r_stream = _host_pack(
        heads, relations, tails, entity_emb, rel_boost_w, rel_rot_w,
        rel_trans_w, ent_bias_w)

    nc = _get_nc()
    cst = np.zeros((P, 8), dtype=np.float32)
    cst[:, 0] = 0.0625
    cst[:, 1] = -0.125
    cst[:, 2] = 0.5
    cst[:, 3] = 1.0
    cst[:, 4] = 1.0 + 1e-6
    cst[:, 5] = 0.0
    in_maps = []
    for i in range(NCORES):
        sl = slice(i * BCORE, (i + 1) * BCORE)
        in_maps.append({"h": np.ascontiguousarray(h_stream[sl]),
                        "t": np.ascontiguousarray(t_stream[sl]),
                        "r": np.ascontiguousarray(r_stream[sl]),
                        "cst": cst})

    res = run_bass_kernel_spmd(nc, in_maps, core_ids=list(range(NCORES)),
                               trace=TRACE)
    LAST_EXEC_NS = res.exec_time_ns
    return np.concatenate([res.results[i]["out"] for i in range(NCORES)])
